# revision 1
# baseline (speedup 1.0000x reference)
"""Graph-transformer layer (masked dense attention + FFN) on 8 trn2 cores.

Sharding (per spec hint): core c handles batch b = c//2 and query rows
[(c%2)*2048, (c%2)*2048+2048) of that batch.  K/V and all weights are
replicated within the 2-core batch group.

Per-core pipeline (fp32 end to end):
  phase A: x blocks -> x^T via PE transpose; K^T [h,n], V [n,h], Q^T [h,q]
           projections.  Biases are folded in exactly as rank-1 accumulate
           matmuls (ones-row x bias-row) into the same PSUM group.
  phase B: per 128-row query tile:
             scores chunk = Q^T.T @ K^T chunk (PSUM, 512 cols)
             P = exp(scores/16)           (ACT, PSUM->SBUF)
             P *= adj; rowsum partials    (DVE tensor_tensor_reduce)
             P^T blocks via PE transpose  -> AV accumulate (PSUM)
             O = AV * (1/rowsum)          (ACT scale-by-AP)
             O^T via PE transpose -> FF1^T = relu(W1^T O^T + b1) -> Y -> DMA
  The softmax skips max-subtraction: scores/16 stays O(5) for any sane
  input so fp32 exp cannot overflow, and softmax is shift-invariant.
  Masked entries are exactly zeroed by the adj multiply, so row sums and
  AV match the reference's -1e9 masking.
"""

import os
from contextlib import ExitStack

import numpy as np

B, N, D, H = 4, 4096, 256, 256
NQ = N // 2  # query rows per core
P = 128  # SBUF partitions
NCHUNK = 512  # scores free-dim chunk = one fp32 PSUM bank
NCORES = 8

_CACHE = {}


def _build():
    import concourse.bass as bass
    import concourse.bacc as bacc
    import concourse.mybir as mybir
    from concourse.tile import TileContext

    f32 = mybir.dt.float32
    i32 = mybir.dt.int32
    AF = mybir.ActivationFunctionType

    n_qt = NQ // P  # 16 query tiles
    n_nb = N // P  # 32 key blocks
    n_ck = N // NCHUNK  # 8 score chunks per row tile
    DT = D // P  # 2 contraction tiles over D
    HT = H // P  # 2 tiles over H

    nc = bacc.Bacc("TRN2", target_bir_lowering=False)

    x_d = nc.dram_tensor("xb", [N, D], f32, kind="ExternalInput").ap()
    xq_d = nc.dram_tensor("xq", [NQ, D], f32, kind="ExternalInput").ap()
    adj_d = nc.dram_tensor("adjs", [NQ, N], i32, kind="ExternalInput").ap()
    w_d = {
        nm: nc.dram_tensor(nm, [256, 256], f32, kind="ExternalInput").ap()
        for nm in ("Wq", "Wk", "Wv", "W1", "W2")
    }
    b_d = {
        nm: nc.dram_tensor(nm, [1, 256], f32, kind="ExternalInput").ap()
        for nm in ("bq", "bk", "bv", "b1", "b2")
    }
    ident_d = nc.dram_tensor("ident_in", [P, P], f32, kind="ExternalInput").ap()
    ones_d = nc.dram_tensor("ones_in", [1, NCHUNK], f32, kind="ExternalInput").ap()
    out_d = nc.dram_tensor("out", [NQ, D], f32, kind="ExternalOutput").ap()

    with ExitStack() as ctx:
        tc = ctx.enter_context(TileContext(nc))
        const = ctx.enter_context(tc.tile_pool(name="const", bufs=1))
        kT_p = ctx.enter_context(tc.tile_pool(name="kT", bufs=1))
        v_p = ctx.enter_context(tc.tile_pool(name="v", bufs=1))
        qT_p = ctx.enter_context(tc.tile_pool(name="qT", bufs=1))
        adj_p = ctx.enter_context(tc.tile_pool(name="adj", bufs=2))
        prow_p = ctx.enter_context(tc.tile_pool(name="prow", bufs=1))
        negm_p = ctx.enter_context(tc.tile_pool(name="negm", bufs=1))
        xin_p = ctx.enter_context(tc.tile_pool(name="xin", bufs=3))
        xtb_p = ctx.enter_context(tc.tile_pool(name="xtb", bufs=3))
        pt_p = ctx.enter_context(tc.tile_pool(name="pt", bufs=4))
        ot_p = ctx.enter_context(tc.tile_pool(name="ot", bufs=3))
        ff_p = ctx.enter_context(tc.tile_pool(name="ff", bufs=3))
        y_p = ctx.enter_context(tc.tile_pool(name="y", bufs=2))
        st_p = ctx.enter_context(tc.tile_pool(name="st", bufs=2))
        tp_ps = ctx.enter_context(tc.tile_pool(name="tp_ps", bufs=3, space="PSUM"))
        mm_ps = ctx.enter_context(tc.tile_pool(name="mm_ps", bufs=4, space="PSUM"))

        # ---- constants ----
        ident = const.tile([P, P], f32)
        nc.sync.dma_start(ident[:], ident_d[:])
        ones = const.tile([1, NCHUNK], f32)
        nc.sync.dma_start(ones[:], ones_d[:])
        w_sb = {}
        for nm in ("Wq", "Wk", "Wv", "W1", "W2"):
            w = const.tile([P, DT, 256], f32, tag=f"w_{nm}")
            for i in range(DT):
                nc.sync.dma_start(w[:, i, :], w_d[nm][i * P : (i + 1) * P, :])
            w_sb[nm] = w
        b_sb = {}
        for nm in ("bq", "bk", "bv", "b1", "b2"):
            bt = const.tile([1, 256], f32, tag=f"b_{nm}")
            nc.sync.dma_start(bt[:], b_d[nm][:])
            b_sb[nm] = bt

        # ---- persistent activations ----
        kT = kT_p.tile([P, HT, N], f32)  # K^T: [h%128, h//128, n]
        v_sb = v_p.tile([P, n_nb, H], f32)  # V: [n%128, n//128, h]
        qT = qT_p.tile([P, HT, NQ], f32)  # Q^T: [h%128, h//128, q]

        def xT_block(src, blk):
            """DMA a 128-row x block and PE-transpose to [d, dt, 128]."""
            xin = xin_p.tile([P, D], f32)
            nc.sync.dma_start(xin[:], src[blk * P : (blk + 1) * P, :])
            xtb = xtb_p.tile([P, DT, P], f32)
            for dt in range(DT):
                ps = tp_ps.tile([P, P], f32, tag="tps")
                nc.tensor.matmul(ps[:], xin[:, dt * P : (dt + 1) * P], ident[:],
                                 start=True, stop=True)
                nc.vector.tensor_copy(xtb[:, dt, :], ps[:])
            return xtb

        def proj_T(dst, dst_sl, wname, bname, xtb):
            """dst[:, ht, dst_sl] = (W^T x + b)^T columns for one 128 block."""
            for ht in range(HT):
                ps = tp_ps.tile([P, P], f32, tag="tps")
                for dt in range(DT):
                    nc.tensor.matmul(
                        ps[:],
                        w_sb[wname][:, dt, ht * P : (ht + 1) * P],
                        xtb[:, dt, :],
                        start=(dt == 0),
                        stop=False,
                    )
                nc.tensor.matmul(
                    ps[:],
                    b_sb[bname][0:1, ht * P : (ht + 1) * P],
                    ones[0:1, 0:P],
                    start=False,
                    stop=True,
                )
                nc.scalar.copy(dst[:, ht, dst_sl], ps[:])

        # ---- phase A ----
        for blk in range(n_nb):
            xtb = xT_block(x_d, blk)
            sl = slice(blk * P, (blk + 1) * P)
            proj_T(kT, sl, "Wk", "bk", xtb)
            psv = mm_ps.tile([P, NCHUNK], f32, tag="mm")
            for dt in range(DT):
                nc.tensor.matmul(
                    psv[:, 0:H],
                    xtb[:, dt, :],
                    w_sb["Wv"][:, dt, :],
                    start=(dt == 0),
                    stop=False,
                )
            nc.tensor.matmul(
                psv[:, 0:H], ones[0:1, 0:P], b_sb["bv"][0:1, :], start=False, stop=True
            )
            nc.scalar.copy(v_sb[:, blk, :], psv[:, 0:H])
        for blk in range(n_qt):
            xtb = xT_block(xq_d, blk)
            proj_T(qT, slice(blk * P, (blk + 1) * P), "Wq", "bq", xtb)

        # ---- phase B ----
        inv_sqrt_h = 1.0 / np.sqrt(np.float32(H))
        for qt in range(n_qt):
            qsl = slice(qt * P, (qt + 1) * P)
            adj_t = adj_p.tile([P, N], i32)
            nc.sync.dma_start(adj_t[:], adj_d[qsl, :])
            prow = prow_p.tile([P, N], f32)
            negm = negm_p.tile([P, N], f32)
            # negmask = (adj - 1) * 1e9 : 0 where edge, -1e9 where masked
            nc.gpsimd.tensor_scalar(
                out=negm[:], in0=adj_t[:], scalar1=-1, scalar2=1e9,
                op0=mybir.AluOpType.add, op1=mybir.AluOpType.mult,
            )
            sums = st_p.tile([P, n_ck], f32, tag="sums")
            for ci in range(n_ck):
                csl = slice(ci * NCHUNK, (ci + 1) * NCHUNK)
                ps = mm_ps.tile([P, NCHUNK], f32, tag="mm")
                for ht in range(HT):
                    nc.tensor.matmul(
                        ps[:],
                        qT[:, ht, qsl],
                        kT[:, ht, csl],
                        start=(ht == 0),
                        stop=(ht == HT - 1),
                    )
                nc.vector.tensor_add(prow[:, csl], ps[:], negm[:, csl])
                nc.scalar.activation(
                    prow[:, csl], prow[:, csl], AF.Exp, scale=inv_sqrt_h,
                    accum_out=sums[:, ci : ci + 1],
                )
            l_all = st_p.tile([P, 1], f32, tag="l_all")
            nc.vector.reduce_sum(l_all[:], sums[:], axis=mybir.AxisListType.X)
            rl = st_p.tile([P, 1], f32, tag="rl")
            nc.vector.reciprocal(rl[:], l_all[:])

            o_ps = mm_ps.tile([P, NCHUNK], f32, tag="mm")
            for blk in range(n_nb):
                tps = tp_ps.tile([P, P], f32, tag="tps")
                nc.tensor.matmul(
                    tps[:], prow[:, blk * P : (blk + 1) * P], ident[:],
                    start=True, stop=True,
                )
                ptb = pt_p.tile([P, P], f32)
                if blk % 2 == 0:
                    nc.vector.tensor_copy(ptb[:], tps[:])
                else:
                    nc.scalar.copy(ptb[:], tps[:])
                nc.tensor.matmul(
                    o_ps[:, 0:H],
                    ptb[:],
                    v_sb[:, blk, :],
                    start=(blk == 0),
                    stop=(blk == n_nb - 1),
                )
            o_sb = ot_p.tile([P, H], f32, tag="o_sb")
            nc.scalar.mul(o_sb[:], o_ps[:, 0:H], rl[:])

            # FFN: FF1^T[h2, q] = relu(W1^T O^T + b1), Y = FF1 W2 + b2
            oT = []
            for ht in range(HT):
                tps = tp_ps.tile([P, P], f32, tag="tps")
                nc.tensor.matmul(tps[:], o_sb[:, ht * P : (ht + 1) * P], ident[:],
                                 start=True, stop=True)
                ot = ot_p.tile([P, P], f32, tag="oT_sb")
                nc.vector.tensor_copy(ot[:], tps[:])
                oT.append(ot)
            ff1 = []
            for ht2 in range(HT):
                fps = tp_ps.tile([P, P], f32, tag="tps")
                for ht in range(HT):
                    nc.tensor.matmul(
                        fps[:],
                        w_sb["W1"][:, ht, ht2 * P : (ht2 + 1) * P],
                        oT[ht][:],
                        start=(ht == 0),
                        stop=False,
                    )
                nc.tensor.matmul(
                    fps[:],
                    b_sb["b1"][0:1, ht2 * P : (ht2 + 1) * P],
                    ones[0:1, 0:P],
                    start=False,
                    stop=True,
                )
                ff = ff_p.tile([P, P], f32)
                nc.scalar.activation(ff[:], fps[:], AF.Relu)
                ff1.append(ff)
            y_ps = mm_ps.tile([P, NCHUNK], f32, tag="mm")
            for ht2 in range(HT):
                nc.tensor.matmul(
                    y_ps[:, 0:D],
                    ff1[ht2][:],
                    w_sb["W2"][:, ht2, :],
                    start=(ht2 == 0),
                    stop=False,
                )
            nc.tensor.matmul(
                y_ps[:, 0:D], ones[0:1, 0:P], b_sb["b2"][0:1, :], start=False,
                stop=True,
            )
            y_sb = y_p.tile([P, D], f32)
            nc.scalar.copy(y_sb[:], y_ps[:, 0:D])
            nc.sync.dma_start(out_d[qsl, :], y_sb[:])

    return nc


def _get_nc():
    if "nc" not in _CACHE:
        nc = _build()
        nc.finalize()  # Bacc: splits multi-sem waits to satisfy HW 1-wait limit
        _CACHE["nc"] = nc
    return _CACHE["nc"]


def kernel(x, adj, Wq, bq, Wk, bk, Wv, bv, W1, b1, W2, b2):
    from concourse.bass_utils import run_bass_kernel_spmd

    x = np.ascontiguousarray(np.asarray(x, dtype=np.float32))
    adj = np.ascontiguousarray(np.asarray(adj, dtype=np.int32))
    weights = {
        "Wq": np.ascontiguousarray(np.asarray(Wq, np.float32)),
        "Wk": np.ascontiguousarray(np.asarray(Wk, np.float32)),
        "Wv": np.ascontiguousarray(np.asarray(Wv, np.float32)),
        "W1": np.ascontiguousarray(np.asarray(W1, np.float32)),
        "W2": np.ascontiguousarray(np.asarray(W2, np.float32)),
        "bq": np.ascontiguousarray(np.asarray(bq, np.float32).reshape(1, 256)),
        "bk": np.ascontiguousarray(np.asarray(bk, np.float32).reshape(1, 256)),
        "bv": np.ascontiguousarray(np.asarray(bv, np.float32).reshape(1, 256)),
        "b1": np.ascontiguousarray(np.asarray(b1, np.float32).reshape(1, 256)),
        "b2": np.ascontiguousarray(np.asarray(b2, np.float32).reshape(1, 256)),
    }
    nc = _get_nc()
    in_maps = []
    for c in range(NCORES):
        b, half = c // 2, c % 2
        q0 = half * NQ
        m = {
            "xb": x[b],
            "xq": np.ascontiguousarray(x[b, q0 : q0 + NQ]),
            "adjs": np.ascontiguousarray(adj[b, q0 : q0 + NQ]),
        }
        m.update(weights)
        m["ident_in"] = np.eye(P, dtype=np.float32)
        m["ones_in"] = np.ones((1, NCHUNK), dtype=np.float32)
        in_maps.append(m)
    global _last_in_maps
    _last_in_maps = in_maps
    res = run_bass_kernel_spmd(nc, in_maps, list(range(NCORES)))
    out = np.empty((B, N, D), dtype=np.float32)
    for c in range(NCORES):
        b, half = c // 2, c % 2
        q0 = half * NQ
        out[b, q0 : q0 + NQ] = res.results[c]["out"]
    return out



# revision 4
# speedup vs baseline: 4.1691x; 4.1691x over previous
"""Graph-transformer layer (masked dense attention + FFN) on 8 trn2 cores.

Sharding (per spec hint): core c handles batch b = c//2 and query rows
[(c%2)*2048, (c%2)*2048+2048) of that batch.  K/V and all weights are
replicated within the 2-core batch group.

All matmuls run in bf16 (fp32 PSUM accumulation); empirically this lands
the full pipeline at ~5e-3 relative error (fp8 variants measured >2e-2).

Per-core pipeline:
  phase A: xT arrives pre-transposed from the host (bf16), so projections
           need no PE transposes:
             K^T[h,n] = Wk^T xT  (ACT evict adds bk per-partition)
             Q^T[h,q] = Wq^T xT[:, q-cols]  (bq likewise)
             V[n,h]   = xT^T Wv  (no bias: bv folds into b1' = b1 + bv@W1
                                  on the host, exact because attn rows sum
                                  to 1); V is stored with a 257th column of
                                  ones so the AV matmul emits row sums free.
  phase B: scores are computed TRANSPOSED, S^T[n,q] = K^T.T @ Q^T, per
           (128-n-block, 512-q-chunk) PSUM tile:
             ex = exp(S^T/16)      (ACT, skips max-subtraction: |s|/16 < ~3)
             P^T = ex * adjT       (DVE; adjT 0/1 bf16 from host, zeroing
                                    masked entries exactly)
             AV[q, 0:257] += P^T_colblk.T @ V_aug  (4 q-subblock accums)
           After 32 n-blocks: O = AV[:,0:256] * (1/AV[:,256]); FFN computed
           from O^T (PE transpose): FF1^T = relu(W1^T O^T + b1') on DVE
           (tensor_scalar add-bias + max 0), Y = FF1 W2 + b2 -> DMA out.
"""

import os
from contextlib import ExitStack

import numpy as np

B, N, D, H = 4, 4096, 256, 256
NQ = N // 2  # query rows per core
P = 128  # SBUF partitions
QCHUNK = 512  # scores free-dim chunk = one fp32 PSUM bank
NCORES = 8

_CACHE = {}


def _build():
    import concourse.bass as bass
    import concourse.bacc as bacc
    import concourse.mybir as mybir
    from concourse.tile import TileContext

    f32 = mybir.dt.float32
    bf16 = mybir.dt.bfloat16
    AF = mybir.ActivationFunctionType
    ALU = mybir.AluOpType

    n_nb = N // P  # 32 key blocks
    n_ck = NQ // QCHUNK  # 4 query chunks per core
    n_qs = QCHUNK // P  # 4 query subblocks per chunk
    DT = D // P  # 2 contraction tiles over D
    HT = H // P  # 2 tiles over H

    nc = bacc.Bacc("TRN2", target_bir_lowering=False)

    xT_d = nc.dram_tensor("xT", [D, N], bf16, kind="ExternalInput").ap()
    adjT_d = nc.dram_tensor("adjT", [N, NQ], bf16, kind="ExternalInput").ap()
    w_d = {
        nm: nc.dram_tensor(nm, [256, 256], bf16, kind="ExternalInput").ap()
        for nm in ("Wq", "Wk", "Wv", "W1", "W2")
    }
    bqT_d = nc.dram_tensor("bqT", [P, HT], f32, kind="ExternalInput").ap()
    bkT_d = nc.dram_tensor("bkT", [P, HT], f32, kind="ExternalInput").ap()
    b1pT_d = nc.dram_tensor("b1pT", [P, HT], f32, kind="ExternalInput").ap()
    b2r_d = nc.dram_tensor("b2r", [1, 256], bf16, kind="ExternalInput").ap()
    identf_d = nc.dram_tensor("identf", [P, P], f32, kind="ExternalInput").ap()
    ones1_d = nc.dram_tensor("ones1", [1, P], bf16, kind="ExternalInput").ap()
    out_d = nc.dram_tensor("out", [NQ, D], f32, kind="ExternalOutput").ap()

    inv_sqrt_h = 1.0 / np.sqrt(np.float32(H))

    with ExitStack() as ctx:
        tc = ctx.enter_context(TileContext(nc))
        const = ctx.enter_context(tc.tile_pool(name="const", bufs=1))
        kT_p = ctx.enter_context(tc.tile_pool(name="kT", bufs=1))
        qT_p = ctx.enter_context(tc.tile_pool(name="qT", bufs=1))
        v_p = ctx.enter_context(tc.tile_pool(name="v", bufs=1))
        xT_p = ctx.enter_context(tc.tile_pool(name="xT", bufs=1))
        adj_p = ctx.enter_context(tc.tile_pool(name="adj", bufs=4))
        ex_p = ctx.enter_context(tc.tile_pool(name="ex", bufs=3))
        pt_p = ctx.enter_context(tc.tile_pool(name="pt", bufs=3))
        o_p = ctx.enter_context(tc.tile_pool(name="o", bufs=2))
        oT_p = ctx.enter_context(tc.tile_pool(name="oT", bufs=2))
        ff1_p = ctx.enter_context(tc.tile_pool(name="ff1", bufs=2))
        y_p = ctx.enter_context(tc.tile_pool(name="y", bufs=2))
        st_p = ctx.enter_context(tc.tile_pool(name="st", bufs=2))
        sc_ps = ctx.enter_context(tc.tile_pool(name="sc_ps", bufs=2, space="PSUM"))
        av_ps = ctx.enter_context(tc.tile_pool(name="av_ps", bufs=4, space="PSUM"))
        mi_ps = ctx.enter_context(tc.tile_pool(name="mi_ps", bufs=2, space="PSUM"))

        # ---- constants ----
        identf = const.tile([P, P], f32)
        nc.sync.dma_start(identf[:], identf_d[:])
        ones1 = const.tile([1, P], bf16)
        nc.sync.dma_start(ones1[:], ones1_d[:])
        b2r = const.tile([1, 256], bf16)
        nc.sync.dma_start(b2r[:], b2r_d[:])
        w_sb = {}
        for nm in ("Wq", "Wk", "Wv", "W1", "W2"):
            w = const.tile([P, DT, 256], bf16, tag=f"w_{nm}")
            for i in range(DT):
                nc.sync.dma_start(w[:, i, :], w_d[nm][i * P : (i + 1) * P, :])
            w_sb[nm] = w
        bqT = const.tile([P, HT], f32, tag="bqT")
        nc.sync.dma_start(bqT[:], bqT_d[:])
        bkT = const.tile([P, HT], f32, tag="bkT")
        nc.sync.dma_start(bkT[:], bkT_d[:])
        b1pT = const.tile([P, HT], f32, tag="b1pT")
        nc.sync.dma_start(b1pT[:], b1pT_d[:])

        # ---- phase A: projections (no PE transposes; xT comes from host) --
        xT = xT_p.tile([P, DT, N], bf16)
        for dt in range(DT):
            nc.sync.dma_start(xT[:, dt, :], xT_d[dt * P : (dt + 1) * P, :])

        kT = kT_p.tile([P, HT, N], bf16)  # K^T: [h%128, h//128, n]
        qT = qT_p.tile([P, HT, NQ], bf16)  # Q^T: [h%128, h//128, q]
        v_sb = v_p.tile([P, n_nb, 257], bf16)  # V_aug: [n%128, n//128, h|ones]
        nc.vector.memset(v_sb[:, :, 256:257], 1.0)

        q0 = 0  # query columns of xT handled by this core are supplied
        # pre-sliced on the host: xT already holds the full batch row block;
        # Q uses columns [qcol0, qcol0+NQ) selected by a runtime-constant
        # offset baked per-core? No -- SPMD: all cores run the same program,
        # so the host instead rotates xT columns so that this core's query
        # rows are ALWAYS columns [0, NQ).  (See kernel(): xTq layout.)

        # K^T and Q^T: out[h-block, n-cols] = sum_dt Wx[d, h].T @ xT[d, n]
        evict_flip = [0]

        def evict(dst, src, bias=None):
            """Alternate psum->sbuf evictions between ACT and DVE."""
            if evict_flip[0] % 2 == 0:
                if bias is None:
                    nc.scalar.copy(dst, src)
                else:
                    nc.scalar.activation(dst, src, AF.Identity, bias=bias)
            else:
                if bias is None:
                    nc.vector.tensor_copy(dst, src)
                else:
                    nc.vector.tensor_scalar_add(dst, src, bias)
            evict_flip[0] += 1

        for ht in range(HT):
            hsl = slice(ht * P, (ht + 1) * P)
            for ck in range(N // QCHUNK):
                csl = slice(ck * QCHUNK, (ck + 1) * QCHUNK)
                ps = sc_ps.tile([P, QCHUNK], f32, tag="sc")
                for dt in range(DT):
                    nc.tensor.matmul(
                        ps[:],
                        w_sb["Wk"][:, dt, hsl],
                        xT[:, dt, csl],
                        start=(dt == 0),
                        stop=(dt == DT - 1),
                    )
                evict(kT[:, ht, csl], ps[:], bias=bkT[:, ht : ht + 1])
        for ht in range(HT):
            hsl = slice(ht * P, (ht + 1) * P)
            for ck in range(n_ck):
                csl = slice(ck * QCHUNK, (ck + 1) * QCHUNK)
                ps = sc_ps.tile([P, QCHUNK], f32, tag="sc")
                for dt in range(DT):
                    nc.tensor.matmul(
                        ps[:],
                        w_sb["Wq"][:, dt, hsl],
                        xT[:, dt, csl],
                        start=(dt == 0),
                        stop=(dt == DT - 1),
                    )
                evict(qT[:, ht, csl], ps[:], bias=bqT[:, ht : ht + 1])
        for blk in range(n_nb):
            bsl = slice(blk * P, (blk + 1) * P)
            ps = av_ps.tile([P, QCHUNK], f32, tag="av")
            for dt in range(DT):
                nc.tensor.matmul(
                    ps[:, 0:256],
                    xT[:, dt, bsl],
                    w_sb["Wv"][:, dt, :],
                    start=(dt == 0),
                    stop=(dt == DT - 1),
                )
            evict(v_sb[:, blk, 0:256], ps[:, 0:256])

        # ---- phase B: attention + FFN per 512-query chunk ----
        for ck in range(n_ck):
            csl = slice(ck * QCHUNK, (ck + 1) * QCHUNK)
            avs = [
                av_ps.tile([P, QCHUNK], f32, tag="av", name="av") for _ in range(n_qs)
            ]
            for blk in range(n_nb):
                bsl = slice(blk * P, (blk + 1) * P)
                adj_t = adj_p.tile([P, QCHUNK], bf16)
                nc.sync.dma_start(adj_t[:], adjT_d[bsl, csl])
                ps = sc_ps.tile([P, QCHUNK], f32, tag="sc")
                for ht in range(HT):
                    nc.tensor.matmul(
                        ps[:],
                        kT[:, ht, bsl],
                        qT[:, ht, csl],
                        start=(ht == 0),
                        stop=(ht == HT - 1),
                    )
                ex = ex_p.tile([P, QCHUNK], bf16)
                nc.scalar.activation(ex[:], ps[:], AF.Exp, scale=inv_sqrt_h)
                pt = pt_p.tile([P, QCHUNK], bf16)
                nc.vector.tensor_mul(pt[:], ex[:], adj_t[:])
                for qs in range(n_qs):
                    nc.tensor.matmul(
                        avs[qs][:, 0:257],
                        pt[:, qs * P : (qs + 1) * P],
                        v_sb[:, blk, :],
                        start=(blk == 0),
                        stop=(blk == n_nb - 1),
                    )
            for qs in range(n_qs):
                av = avs[qs]
                rl = st_p.tile([P, 1], f32, tag="rl")
                nc.vector.reciprocal(rl[:], av[:, 256:257])
                o_sb = o_p.tile([P, 256], f32)
                nc.vector.tensor_scalar_mul(o_sb[:], av[:, 0:256], rl[:])
                # O^T via PE transpose (f32), then FFN entirely transposed.
                tp = mi_ps.tile([P, QCHUNK], f32, tag="mi")
                oT = oT_p.tile([P, HT, P], bf16)
                for ht in range(HT):
                    nc.tensor.transpose(
                        tp[:, ht * P : (ht + 1) * P],
                        o_sb[:, ht * P : (ht + 1) * P],
                        identf[:],
                    )
                    nc.vector.tensor_copy(oT[:, ht, :], tp[:, ht * P : (ht + 1) * P])
                fp = mi_ps.tile([P, QCHUNK], f32, tag="mi")
                ff1 = ff1_p.tile([P, HT, P], bf16)
                for h2 in range(HT):
                    fsl = slice(h2 * P, (h2 + 1) * P)
                    for ht in range(HT):
                        nc.tensor.matmul(
                            fp[:, fsl],
                            w_sb["W1"][:, ht, h2 * P : (h2 + 1) * P],
                            oT[:, ht, :],
                            start=(ht == 0),
                            stop=(ht == HT - 1),
                        )
                    # relu(x + b1') on DVE: (x add b1') max 0
                    nc.vector.tensor_scalar(
                        out=ff1[:, h2, :],
                        in0=fp[:, fsl],
                        scalar1=b1pT[:, h2 : h2 + 1],
                        scalar2=0.0,
                        op0=ALU.add,
                        op1=ALU.max,
                    )
                yp = mi_ps.tile([P, QCHUNK], f32, tag="mi")
                for h2 in range(HT):
                    nc.tensor.matmul(
                        yp[:, 0:256],
                        ff1[:, h2, :],
                        w_sb["W2"][:, h2, :],
                        start=(h2 == 0),
                        stop=False,
                    )
                nc.tensor.matmul(
                    yp[:, 0:256], ones1[0:1, :], b2r[0:1, :], start=False, stop=True
                )
                y_sb = y_p.tile([P, 256], f32)
                nc.vector.tensor_copy(y_sb[:], yp[:, 0:256])
                row0 = ck * QCHUNK + qs * P
                nc.sync.dma_start(out_d[row0 : row0 + P, :], y_sb[:])

    return nc


def _get_nc():
    if "nc" not in _CACHE:
        nc = _build()
        nc.finalize()  # Bacc: splits multi-sem waits to satisfy HW 1-wait limit
        _CACHE["nc"] = nc
    return _CACHE["nc"]


def _host_inputs(x, adj, Wq, bq, Wk, bk, Wv, bv, W1, b1, W2, b2):
    import ml_dtypes

    bf16 = ml_dtypes.bfloat16

    x = np.asarray(x, dtype=np.float32)
    adj = np.asarray(adj)
    f32w = {
        "Wq": np.asarray(Wq, np.float32),
        "Wk": np.asarray(Wk, np.float32),
        "Wv": np.asarray(Wv, np.float32),
        "W1": np.asarray(W1, np.float32),
        "W2": np.asarray(W2, np.float32),
    }
    bq = np.asarray(bq, np.float32).reshape(256)
    bk = np.asarray(bk, np.float32).reshape(256)
    bv = np.asarray(bv, np.float32).reshape(256)
    b1 = np.asarray(b1, np.float32).reshape(256)
    b2 = np.asarray(b2, np.float32).reshape(256)
    b1p = b1 + bv @ f32w["W1"]  # exact: attn rows sum to 1

    weights = {nm: np.ascontiguousarray(w.astype(bf16)) for nm, w in f32w.items()}
    weights["bqT"] = np.ascontiguousarray(bq.reshape(2, 128).T.astype(np.float32))
    weights["bkT"] = np.ascontiguousarray(bk.reshape(2, 128).T.astype(np.float32))
    weights["b1pT"] = np.ascontiguousarray(b1p.reshape(2, 128).T.astype(np.float32))
    weights["b2r"] = np.ascontiguousarray(b2.reshape(1, 256).astype(bf16))
    weights["identf"] = np.eye(P, dtype=np.float32)
    weights["ones1"] = np.ones((1, P), dtype=bf16)

    in_maps = []
    for c in range(NCORES):
        b, half = c // 2, c % 2
        qcol0 = half * NQ
        # Rotate xT columns so this core's query rows occupy columns
        # [0, NQ) -- the SPMD program always reads Q from there, while
        # K/V still see all N columns (order of n is irrelevant as long
        # as adjT rows are permuted identically).
        xb = x[b]
        perm = np.r_[qcol0 : qcol0 + NQ, 0:qcol0, qcol0 + NQ : N]
        xTc = np.ascontiguousarray(xb[perm].T.astype(bf16))
        adjc = adj[b, qcol0 : qcol0 + NQ, :]  # [NQ, N] int
        adjTc = np.ascontiguousarray(adjc[:, perm].T.astype(bf16))
        m = {"xT": xTc, "adjT": adjTc}
        m.update(weights)
        in_maps.append(m)
    return in_maps


def kernel(x, adj, Wq, bq, Wk, bk, Wv, bv, W1, b1, W2, b2):
    from concourse.bass_utils import run_bass_kernel_spmd

    nc = _get_nc()
    in_maps = _host_inputs(x, adj, Wq, bq, Wk, bk, Wv, bv, W1, b1, W2, b2)
    global _last_in_maps
    _last_in_maps = in_maps
    res = run_bass_kernel_spmd(nc, in_maps, list(range(NCORES)))
    out = np.empty((B, N, D), dtype=np.float32)
    for c in range(NCORES):
        b, half = c // 2, c % 2
        q0 = half * NQ
        out[b, q0 : q0 + NQ] = res.results[c]["out"]
    return out
